# revision 15
# baseline (speedup 1.0000x reference)
"""BitLinear (2-bit packed ternary-ish weights) on 8 Trainium2 NeuronCores.

y = round_int8(x * 127/amax_row) @ unpack(weight_packed).T / (ws * 127/amax_row) + bias

Sharding: data-parallel over the flattened token dim (16384 rows -> 2048
rows/core). The packed weight is tiny; it is unpacked host-side to bf16
(values {-1,0,1,2}, exact in bf16) and replicated to all cores.

On-device math is bit-faithful to the jax reference:
  - absmax reduce + clip:   exact f32 ops
  - scale = 127/amax:       DVE IEEE reciprocal + multiply
  - xq = round(x*scale):    DVE two-stage (mult, +1.5*2^23) == f32 mult-round
                            then round-half-to-even; ints <=127 exact in bf16
  - matmul:                 bf16 PE with f32 PSUM accumulation — products and
                            partial sums are integers < 2^24 => exact
  - dequant+bias:           fused DVE (psum * rden) + bias, f32

Schedule (vs the first-pass kernel at 310us):
  - xq transpose moved off the PE onto the DMA xbar (dma_start_transpose,
    one [128, 16, 128] bf16 SBUF->SBUF transfer per m-tile) — the PE runs
    nothing but the 64 main matmuls per tile.
  - k-inner matmul order per output half: each 2-bank PSUM tile finishes
    while the other half computes, so drains spread instead of bunching.
  - PSUM double-buffered: 2 halves x 2 bufs x 2 banks = all 8 banks.
  - dequant + bias fused into one DVE scalar_tensor_tensor pass per half,
    which also performs the PSUM->SBUF drain.
  - y stores issued from the ACT engine's DMA queue, x loads + transposes
    from the sync engine's queue.
"""

from contextlib import ExitStack

import numpy as np
import ml_dtypes

import concourse.bass as bass
import concourse.mybir as mybir
import concourse.tile as tile
from concourse import bacc
from concourse.bass_utils import run_bass_kernel_spmd
from concourse.masks import make_identity

P = 128
D = 2048               # in_features
O = 2048               # out_features (4 * 512 packed rows)
N_CORES = 8
B, S = 4, 4096
M_TOTAL = B * S        # 16384
M_CORE = M_TOTAL // N_CORES   # 2048
NK = D // P            # 16 contraction blocks
OH = O // 2            # 1024: one PSUM tile (2 banks) per output half
MAGIC = 12582912.0     # 1.5 * 2^23 — f32 add forces round-half-to-even to int
QP = 127.0


def build_nc(m_core=M_CORE, repeats=1, variant="main"):
    """variant: 'main' | 'mmonly' (pure matmul stream, constant operands)
    | 'noquant' (constant xq, no x load/quant) | 'petrans' (PE-transpose
    path instead of the DMA xbar) | 'mmsame' (mmonly with one shared
    stationary -> 1 LDW/tile, clock probe) | 'mm256' (mmonly with 256-col
    matmuls, per-instruction overhead probe)."""
    m_tiles = m_core // P
    nc = bacc.Bacc(None)
    x = nc.declare_dram_parameter("x", [m_core, D], mybir.dt.float32, isOutput=False)
    wT = nc.declare_dram_parameter("wT", [D, O], mybir.dt.bfloat16, isOutput=False)
    bias = nc.declare_dram_parameter("bias", [O], mybir.dt.float32, isOutput=False)
    ws = nc.declare_dram_parameter("ws", [1], mybir.dt.float32, isOutput=False)
    y = nc.declare_dram_parameter("y", [m_core, O], mybir.dt.float32, isOutput=True)

    with ExitStack() as ctx:
        tc = ctx.enter_context(tile.TileContext(nc))
        # bufs of every per-tile pool divide the tile count (16) so the
        # rotated For_i pipeline's end-of-body heads land on the same slots
        # the start-of-body tails reference.
        consts = ctx.enter_context(tc.tile_pool(name="consts", bufs=1))
        xpool = ctx.enter_context(tc.tile_pool(name="xin", bufs=4))
        qpool = ctx.enter_context(tc.tile_pool(name="quant", bufs=4))
        tppool = ctx.enter_context(tc.tile_pool(name="xqt", bufs=4))
        spool = ctx.enter_context(tc.tile_pool(name="stats", bufs=4))
        opool = ctx.enter_context(tc.tile_pool(name="yout", bufs=4))
        psum_bufs = 1 if variant == "petrans" else 2
        psy = ctx.enter_context(tc.tile_pool(name="psy", bufs=psum_bufs, space="PSUM"))

        bias_sb = consts.tile([P, O], mybir.dt.float32)
        nc.sync.dma_start(bias_sb[:], bias[None, :].to_broadcast((P, O)))
        ws_sb = consts.tile([P, 1], mybir.dt.float32)
        nc.sync.dma_start(ws_sb[:], ws[None, :].to_broadcast((P, 1)))
        w_sb = consts.tile([P, NK, O], mybir.dt.bfloat16)
        nc.sync.dma_start(w_sb[:], wT.rearrange("(k p) o -> p k o", p=P))
        if variant == "petrans":
            ident = consts.tile([P, P], mybir.dt.bfloat16)
            make_identity(nc, ident[:])
            pst = ctx.enter_context(tc.tile_pool(name="pst", bufs=3, space="PSUM"))

        x3 = x.rearrange("(t p) d -> t p d", p=P)
        y3 = y.rearrange("(t p) o -> t p o", p=P)

        def emit_head(t):
            """x load, activation quant, transpose issue for tile t.
            Returns (xqT, rden)."""
            if variant in ("mmonly", "noquant", "mmsame", "mm256"):
                xq = qpool.tile([P, D], mybir.dt.bfloat16, tag="xq")
                nc.vector.memset(xq[:], 1.0)
                rden = spool.tile([P, 1], mybir.dt.float32, tag="rden")
                nc.vector.memset(rden[:], 1.0)
            else:
                xt = xpool.tile([P, D], mybir.dt.float32, tag="xin")
                nc.sync.dma_start(xt[:], x3[t])

                amax = spool.tile([P, 1], mybir.dt.float32, tag="amax")
                nc.vector.reduce_max(
                    amax[:], xt[:], axis=mybir.AxisListType.X,
                    apply_absolute_value=True,
                )
                nc.vector.tensor_scalar_max(amax[:], amax[:], 1e-5)
                # scl = 127 * (1/amax); HW reciprocal is IEEE 1/x, so scl
                # is within 1 ulp of the reference's fl(127/amax)
                ramax = spool.tile([P, 1], mybir.dt.float32, tag="ramax")
                nc.vector.reciprocal(ramax[:], amax[:])
                scl = spool.tile([P, 1], mybir.dt.float32, tag="scl")
                nc.vector.tensor_scalar_mul(scl[:], ramax[:], QP)
                den = spool.tile([P, 1], mybir.dt.float32, tag="den")
                nc.vector.tensor_tensor(
                    den[:], ws_sb[:], scl[:], mybir.AluOpType.mult
                )
                rden = spool.tile([P, 1], mybir.dt.float32, tag="rden")
                nc.vector.reciprocal(rden[:], den[:])

                # xq = round_half_even(x * scale), exact ints in bf16
                t1 = qpool.tile([P, D], mybir.dt.float32, tag="t1", bufs=2)
                nc.vector.tensor_scalar(
                    t1[:], xt[:], scl[:], MAGIC,
                    op0=mybir.AluOpType.mult, op1=mybir.AluOpType.add,
                )
                xq = qpool.tile([P, D], mybir.dt.bfloat16, tag="xq")
                nc.scalar.activation(
                    xq[:], t1[:], mybir.ActivationFunctionType.Copy,
                    bias=-MAGIC, scale=1.0,
                )

            # xqT[d, k, m] = xq[m, k*128 + d]
            xqT = tppool.tile([P, NK, P], mybir.dt.bfloat16, tag="xqT")
            if variant == "petrans":
                for g in range(NK // 4):
                    pt = pst.tile([P, 4 * P], mybir.dt.float32,
                                  tag="pst", name=f"pst{g}")
                    for kk in range(4):
                        nc.tensor.matmul(
                            pt[:, bass.ts(kk, P)],
                            xq[:, bass.ts(g * 4 + kk, P)], ident[:],
                            start=True, stop=True,
                        )
                    nc.scalar.copy(
                        xqT[:, g * 4:(g + 1) * 4, :],
                        pt[:].rearrange("p (a b) -> p a b", b=P))
            elif variant in ("mmonly", "mmsame", "mm256"):
                nc.vector.memset(xqT[:], 1.0)
            else:
                # issued on ACT (hwdge) directly after the xq convert, so the
                # transfer starts as soon as xq lands and rides its own queue
                nc.scalar.dma_start_transpose(xqT[:], xq[:])
            return xqT, rden

        def emit_tail(t, xqT, rden):
            # y[m, o] = (sum_d xqT[d, m] * w[d, o]) * rden[m] + bias[o].
            # k outermost: the 4 matmuls per k share the stationary xqT[:,k,:],
            # so after the LDW dedup pass only 16 weight loads remain per tile.
            ys = [
                psy.tile([P, OH], mybir.dt.float32, tag=f"ps{j}", name=f"ps{j}")
                for j in range(2)
            ]
            nsub = 4 if variant == "mm256" else 2
            sub = OH // nsub
            if variant == "bankdwell":
                # 16 consecutive accumulating matmuls into one bank slice
                for j in range(2):
                    for s in range(2):
                        for k in range(NK):
                            nc.tensor.matmul(
                                ys[j][:, bass.ts(s, sub)],
                                xqT[:, k, :],
                                w_sb[:, k, j * OH + s * sub:
                                     j * OH + (s + 1) * sub],
                                start=(k == 0), stop=(k == NK - 1),
                            )
            else:
                for k in range(NK):
                    kw = 0 if variant == "mmsame" else k
                    for j in range(2):
                        for s in range(nsub):
                            nc.tensor.matmul(
                                ys[j][:, bass.ts(s, sub)],
                                xqT[:, kw, :],
                                w_sb[:, k, j * OH + s * sub:
                                     j * OH + (s + 1) * sub],
                                start=(k == 0), stop=(k == NK - 1),
                            )
            for j in range(2):
                yt = opool.tile([P, OH], mybir.dt.float32,
                                tag=f"yt{j}", name=f"yt{j}")
                nc.vector.scalar_tensor_tensor(
                    yt[:], ys[j][:], rden[:], bias_sb[:, bass.ts(j, OH)],
                    op0=mybir.AluOpType.mult, op1=mybir.AluOpType.add,
                )
                nc.scalar.dma_start(y3[t][:, bass.ts(j, OH)], yt[:])

        def body(_iv=None):
            # 2-deep software pipeline: tile t's head (load/quant/transpose)
            # is emitted two tails early, so the serial chain
            # x-DMA -> amax -> quant -> xbar-transpose has two full matmul
            # tails of slack before the PE needs xqT(t).
            from collections import deque
            pending = deque()
            for t in range(m_tiles):
                pending.append((t, *emit_head(t)))
                if len(pending) > 2:
                    emit_tail(*pending.popleft())
            while pending:
                emit_tail(*pending.popleft())

        if repeats == 1:
            body()
        elif repeats > 1:
            # hardware loop: constant program size for any repeat count
            with tc.For_i(0, repeats, 1):
                body()
    _dedup_ldweights(nc)
    nc.finalize()
    return nc


def _dedup_ldweights(nc):
    """Drop InstLdweights whose stationary operand is identical to the one
    already loaded by the previous PE weight load. The PE array keeps the
    stationary operand across matmuls (InstMatmult here are emitted with
    ldweights=False), so consecutive matmuls sharing lhsT only need the
    first load. Only sync-free LDWs are dropped, so the semaphore protocol
    is untouched; tracking resets at block boundaries."""
    pe = mybir.EngineType.PE
    for blk in nc.m.functions[0].blocks:
        last_sig = None
        keep = []
        for inst in blk.instructions:
            if getattr(inst, "engine", None) != pe:
                keep.append(inst)
                continue
            if isinstance(inst, mybir.InstLdweights):
                ap = inst.ins[0]
                sig = (
                    ap.memref, ap.offset, tuple(map(tuple, ap.ap)),
                    inst.is_transpose, inst.perf_mode,
                    inst.tile_position, inst.tile_size,
                )
                si = inst.sync_info
                sync_free = si is None or (not si.on_wait and not si.on_update)
                if sig == last_sig and sync_free:
                    continue  # redundant reload — drop
                last_sig = sig
            elif not isinstance(inst, mybir.InstMatmult):
                # unknown PE-engine instruction: be conservative
                last_sig = None
            keep.append(inst)
        if len(keep) != len(blk.instructions):
            blk.instructions[:] = keep


def unpack_weights_host(weight_packed):
    """[512, 2048] int32 packed -> [2048 in, 2048 out] bf16 transposed weight."""
    wp = np.asarray(weight_packed)
    parts = [((wp >> (2 * i)) & 3) for i in range(4)]
    w = np.concatenate(parts, axis=0).astype(np.float32) - 1.0   # [out, in]
    return np.ascontiguousarray(w.T).astype(ml_dtypes.bfloat16)  # [in, out]


_NC_CACHE = {}


def _get_nc():
    if "nc" not in _NC_CACHE:
        _NC_CACHE["nc"] = build_nc()
    return _NC_CACHE["nc"]


def make_in_maps(inputs):
    x = inputs["x"]
    xf = np.ascontiguousarray(np.asarray(x, dtype=np.float32).reshape(M_TOTAL, D))
    wT = unpack_weights_host(inputs["weight_packed"])
    bias_np = np.ascontiguousarray(np.asarray(inputs["bias"], dtype=np.float32))
    ws_np = np.ascontiguousarray(
        np.asarray(inputs["weight_scale"], dtype=np.float32))
    return [
        {
            "x": xf[i * M_CORE:(i + 1) * M_CORE],
            "wT": wT,
            "bias": bias_np,
            "ws": ws_np,
        }
        for i in range(N_CORES)
    ]


def kernel(x, weight_packed, weight_scale, bias):
    in_maps = make_in_maps(
        {"x": x, "weight_packed": weight_packed,
         "weight_scale": weight_scale, "bias": bias})
    res = run_bass_kernel_spmd(_get_nc(), in_maps, list(range(N_CORES))).results
    y = np.concatenate([res[i]["y"] for i in range(N_CORES)], axis=0)
    return np.ascontiguousarray(y.reshape(B, S, O))


# revision 19
# speedup vs baseline: 1.1411x; 1.1411x over previous
"""BitLinear (2-bit packed ternary-ish weights) on 8 Trainium2 NeuronCores.

y = round_int8(x * 127/amax_row) @ unpack(weight_packed).T / (ws * 127/amax_row) + bias

Sharding: data-parallel over the flattened token dim (16384 rows -> 2048
rows/core). The packed weight is tiny; it is unpacked host-side to bf16
(values {-1,0,1,2}, exact in bf16) and replicated to all cores.

On-device math is bit-faithful to the jax reference:
  - absmax reduce + clip:   exact f32 ops
  - scale = 127/amax:       DVE IEEE reciprocal + multiply
  - xq = round(x*scale):    DVE two-stage (mult, +1.5*2^23) == f32 mult-round
                            then round-half-to-even; ints <=127 exact in bf16
  - matmul:                 bf16 PE with f32 PSUM accumulation — products and
                            partial sums are integers < 2^24 => exact
  - dequant+bias:           fused DVE (psum * rden) + bias, f32

Schedule (vs the first-pass kernel at 310us):
  - xq transpose moved off the PE onto the DMA xbar (dma_start_transpose,
    one [128, 16, 128] bf16 SBUF->SBUF transfer per m-tile) — the PE runs
    nothing but the 64 main matmuls per tile.
  - k-inner matmul order per output half: each 2-bank PSUM tile finishes
    while the other half computes, so drains spread instead of bunching.
  - PSUM double-buffered: 2 halves x 2 bufs x 2 banks = all 8 banks.
  - dequant + bias fused into one DVE scalar_tensor_tensor pass per half,
    which also performs the PSUM->SBUF drain.
  - y stores issued from the ACT engine's DMA queue, x loads + transposes
    from the sync engine's queue.
"""

from contextlib import ExitStack

import numpy as np
import ml_dtypes

import concourse.bass as bass
import concourse.mybir as mybir
import concourse.tile as tile
from concourse import bacc
from concourse.bass_utils import run_bass_kernel_spmd
from concourse.masks import make_identity

P = 128
D = 2048               # in_features
O = 2048               # out_features (4 * 512 packed rows)
N_CORES = 8
B, S = 4, 4096
M_TOTAL = B * S        # 16384
M_CORE = M_TOTAL // N_CORES   # 2048
NK = D // P            # 16 contraction blocks
OH = O // 2            # 1024: one PSUM tile (2 banks) per output half
MAGIC = 12582912.0     # 1.5 * 2^23 — f32 add forces round-half-to-even to int
QP = 127.0


def build_nc(m_core=M_CORE, repeats=1, variant="main"):
    """variant: 'main' | 'mmonly' (pure matmul stream, constant operands)
    | 'noquant' (constant xq, no x load/quant) | 'petrans' (PE-transpose
    path instead of the DMA xbar) | 'mmsame' (mmonly with one shared
    stationary -> 1 LDW/tile, clock probe) | 'mm256' (mmonly with 256-col
    matmuls, per-instruction overhead probe)."""
    m_tiles = m_core // P
    nc = bacc.Bacc(None)
    x = nc.declare_dram_parameter("x", [m_core, D], mybir.dt.float32, isOutput=False)
    wT = nc.declare_dram_parameter("wT", [D, O], mybir.dt.bfloat16, isOutput=False)
    bias = nc.declare_dram_parameter("bias", [O], mybir.dt.float32, isOutput=False)
    ws = nc.declare_dram_parameter("ws", [1], mybir.dt.float32, isOutput=False)
    y = nc.declare_dram_parameter("y", [m_core, O], mybir.dt.float32, isOutput=True)

    with ExitStack() as ctx:
        tc = ctx.enter_context(tile.TileContext(nc))
        # bufs of every per-tile pool divide the tile count (16) so the
        # rotated For_i pipeline's end-of-body heads land on the same slots
        # the start-of-body tails reference.
        consts = ctx.enter_context(tc.tile_pool(name="consts", bufs=1))
        xpool = ctx.enter_context(tc.tile_pool(name="xin", bufs=4))
        qpool = ctx.enter_context(tc.tile_pool(name="quant", bufs=4))
        tppool = ctx.enter_context(tc.tile_pool(name="xqt", bufs=4))
        spool = ctx.enter_context(tc.tile_pool(name="stats", bufs=4))
        opool = ctx.enter_context(tc.tile_pool(name="yout", bufs=4))
        psum_bufs = 1 if variant == "petrans" else 2
        psy = ctx.enter_context(tc.tile_pool(name="psy", bufs=psum_bufs, space="PSUM"))

        bias_sb = consts.tile([P, O], mybir.dt.float32)
        nc.sync.dma_start(bias_sb[:], bias[None, :].to_broadcast((P, O)))
        ws_sb = consts.tile([P, 1], mybir.dt.float32)
        nc.sync.dma_start(ws_sb[:], ws[None, :].to_broadcast((P, 1)))
        # wsinv = 1/(127*ws): rden = amax_clipped * wsinv (2 fewer DVE ops
        # per tile than den=ws*scl; rden=1/den; same value to ~1 ulp)
        wsq = consts.tile([P, 1], mybir.dt.float32)
        nc.vector.tensor_scalar_mul(wsq[:], ws_sb[:], QP)
        wsinv = consts.tile([P, 1], mybir.dt.float32)
        nc.vector.reciprocal(wsinv[:], wsq[:])
        w_sb = consts.tile([P, NK, O], mybir.dt.bfloat16)
        nc.sync.dma_start(w_sb[:], wT.rearrange("(k p) o -> p k o", p=P))
        if variant == "petrans":
            ident = consts.tile([P, P], mybir.dt.bfloat16)
            make_identity(nc, ident[:])
            pst = ctx.enter_context(tc.tile_pool(name="pst", bufs=3, space="PSUM"))

        x3 = x.rearrange("(t p) d -> t p d", p=P)
        y3 = y.rearrange("(t p) o -> t p o", p=P)

        def emit_head(t):
            """x load, activation quant, transpose issue for tile t.
            Returns (xqT, rden)."""
            if variant in ("mmonly", "noquant", "mmsame", "mm256"):
                xq = qpool.tile([P, D], mybir.dt.bfloat16, tag="xq")
                nc.vector.memset(xq[:], 1.0)
                rden = spool.tile([P, 1], mybir.dt.float32, tag="rden")
                nc.vector.memset(rden[:], 1.0)
            else:
                xt = xpool.tile([P, D], mybir.dt.float32, tag="xin")
                nc.sync.dma_start(xt[:], x3[t])

                amax = spool.tile([P, 1], mybir.dt.float32, tag="amax")
                nc.vector.reduce_max(
                    amax[:], xt[:], axis=mybir.AxisListType.X,
                    apply_absolute_value=True,
                )
                nc.vector.tensor_scalar_max(amax[:], amax[:], 1e-5)
                # scl = 127 * (1/amax); HW reciprocal is IEEE 1/x, so scl
                # is within 1 ulp of the reference's fl(127/amax)
                ramax = spool.tile([P, 1], mybir.dt.float32, tag="ramax")
                nc.vector.reciprocal(ramax[:], amax[:])
                scl = spool.tile([P, 1], mybir.dt.float32, tag="scl")
                nc.vector.tensor_scalar_mul(scl[:], ramax[:], QP)
                rden = spool.tile([P, 1], mybir.dt.float32, tag="rden")
                nc.vector.tensor_tensor(
                    rden[:], amax[:], wsinv[:], mybir.AluOpType.mult
                )

                # xq = round_half_even(x * scale), exact ints in bf16
                t1 = qpool.tile([P, D], mybir.dt.float32, tag="t1", bufs=2)
                nc.vector.tensor_scalar(
                    t1[:], xt[:], scl[:], MAGIC,
                    op0=mybir.AluOpType.mult, op1=mybir.AluOpType.add,
                )
                xq = qpool.tile([P, D], mybir.dt.bfloat16, tag="xq")
                nc.scalar.activation(
                    xq[:], t1[:], mybir.ActivationFunctionType.Copy,
                    bias=-MAGIC, scale=1.0,
                )

            # xqT[d, k, m] = xq[m, k*128 + d]
            xqT = tppool.tile([P, NK, P], mybir.dt.bfloat16, tag="xqT")
            if variant == "petrans":
                for g in range(NK // 4):
                    pt = pst.tile([P, 4 * P], mybir.dt.float32,
                                  tag="pst", name=f"pst{g}")
                    for kk in range(4):
                        nc.tensor.matmul(
                            pt[:, bass.ts(kk, P)],
                            xq[:, bass.ts(g * 4 + kk, P)], ident[:],
                            start=True, stop=True,
                        )
                    nc.scalar.copy(
                        xqT[:, g * 4:(g + 1) * 4, :],
                        pt[:].rearrange("p (a b) -> p a b", b=P))
            elif variant in ("mmonly", "mmsame", "mm256"):
                nc.vector.memset(xqT[:], 1.0)
            else:
                # issued on ACT (hwdge) directly after the xq convert, so the
                # transfer starts as soon as xq lands and rides its own queue
                nc.scalar.dma_start_transpose(xqT[:], xq[:])
            return xqT, rden

        def emit_tail(t, xqT, rden):
            # y[m, o] = (sum_d xqT[d, m] * w[d, o]) * rden[m] + bias[o].
            # k outermost: the 4 matmuls per k share the stationary xqT[:,k,:],
            # so after the LDW dedup pass only 16 weight loads remain per tile.
            ys = [
                psy.tile([P, OH], mybir.dt.float32, tag=f"ps{j}", name=f"ps{j}")
                for j in range(2)
            ]
            nsub = 4 if variant == "mm256" else 2
            sub = OH // nsub
            if variant == "bankdwell":
                # 16 consecutive accumulating matmuls into one bank slice
                for j in range(2):
                    for s in range(2):
                        for k in range(NK):
                            nc.tensor.matmul(
                                ys[j][:, bass.ts(s, sub)],
                                xqT[:, k, :],
                                w_sb[:, k, j * OH + s * sub:
                                     j * OH + (s + 1) * sub],
                                start=(k == 0), stop=(k == NK - 1),
                            )
            else:
                for k in range(NK):
                    kw = 0 if variant == "mmsame" else k
                    for j in range(2):
                        for s in range(nsub):
                            nc.tensor.matmul(
                                ys[j][:, bass.ts(s, sub)],
                                xqT[:, kw, :],
                                w_sb[:, k, j * OH + s * sub:
                                     j * OH + (s + 1) * sub],
                                start=(k == 0), stop=(k == NK - 1),
                            )
            for j in range(2):
                yt = opool.tile([P, OH], mybir.dt.float32,
                                tag=f"yt{j}", name=f"yt{j}")
                nc.vector.scalar_tensor_tensor(
                    yt[:], ys[j][:], rden[:], bias_sb[:, bass.ts(j, OH)],
                    op0=mybir.AluOpType.mult, op1=mybir.AluOpType.add,
                )
                nc.sync.dma_start(y3[t][:, bass.ts(j, OH)], yt[:])

        DEPTH = 2  # heads lead tails by two tiles

        if repeats == 1:
            # 2-deep software pipeline: tile t's head (load/quant/transpose)
            # is emitted two tails early, so the serial chain
            # x-DMA -> amax -> quant -> xbar-transpose has two full matmul
            # tails of slack before the PE needs xqT(t).
            from collections import deque
            pending = deque()
            for t in range(m_tiles):
                pending.append((t, *emit_head(t)))
                if len(pending) > DEPTH:
                    emit_tail(*pending.popleft())
            while pending:
                emit_tail(*pending.popleft())
        elif repeats > 1:
            # Rotated hardware loop: two heads are primed before For_i, and
            # the body emits head((t+DEPTH) % m_tiles) next to tail(t). The
            # end-of-body heads (tiles 0..DEPTH-1) write the same pool slots
            # the start-of-body tails read (every per-tile pool's bufs
            # divides m_tiles), so each firing re-primes the next and the
            # pipeline crosses iteration boundaries with no PE bubble.
            # Re-loading x[0..DEPTH-1] each firing recomputes identical
            # values, so every iteration's output is correct.
            assert m_tiles % DEPTH == 0
            from collections import deque
            pending = deque(
                (t, *emit_head(t)) for t in range(DEPTH)
            )
            with tc.For_i(0, repeats, 1):
                for t in range(m_tiles):
                    pending.append(
                        ((t + DEPTH) % m_tiles,
                         *emit_head((t + DEPTH) % m_tiles)))
                    emit_tail(*pending.popleft())
    _dedup_ldweights(nc)
    nc.finalize()
    return nc


def _dedup_ldweights(nc):
    """Drop InstLdweights whose stationary operand is identical to the one
    already loaded by the previous PE weight load. The PE array keeps the
    stationary operand across matmuls (InstMatmult here are emitted with
    ldweights=False), so consecutive matmuls sharing lhsT only need the
    first load. Only sync-free LDWs are dropped, so the semaphore protocol
    is untouched; tracking resets at block boundaries."""
    pe = mybir.EngineType.PE
    for blk in nc.m.functions[0].blocks:
        last_sig = None
        keep = []
        for inst in blk.instructions:
            if getattr(inst, "engine", None) != pe:
                keep.append(inst)
                continue
            if isinstance(inst, mybir.InstLdweights):
                ap = inst.ins[0]
                sig = (
                    ap.memref, ap.offset, tuple(map(tuple, ap.ap)),
                    inst.is_transpose, inst.perf_mode,
                    inst.tile_position, inst.tile_size,
                )
                si = inst.sync_info
                sync_free = si is None or (not si.on_wait and not si.on_update)
                if sig == last_sig and sync_free:
                    continue  # redundant reload — drop
                last_sig = sig
            elif not isinstance(inst, mybir.InstMatmult):
                # unknown PE-engine instruction: be conservative
                last_sig = None
            keep.append(inst)
        if len(keep) != len(blk.instructions):
            blk.instructions[:] = keep


def unpack_weights_host(weight_packed):
    """[512, 2048] int32 packed -> [2048 in, 2048 out] bf16 transposed weight."""
    wp = np.asarray(weight_packed)
    parts = [((wp >> (2 * i)) & 3) for i in range(4)]
    w = np.concatenate(parts, axis=0).astype(np.float32) - 1.0   # [out, in]
    return np.ascontiguousarray(w.T).astype(ml_dtypes.bfloat16)  # [in, out]


_NC_CACHE = {}


def _get_nc():
    if "nc" not in _NC_CACHE:
        _NC_CACHE["nc"] = build_nc()
    return _NC_CACHE["nc"]


def make_in_maps(inputs):
    x = inputs["x"]
    xf = np.ascontiguousarray(np.asarray(x, dtype=np.float32).reshape(M_TOTAL, D))
    wT = unpack_weights_host(inputs["weight_packed"])
    bias_np = np.ascontiguousarray(np.asarray(inputs["bias"], dtype=np.float32))
    ws_np = np.ascontiguousarray(
        np.asarray(inputs["weight_scale"], dtype=np.float32))
    return [
        {
            "x": xf[i * M_CORE:(i + 1) * M_CORE],
            "wT": wT,
            "bias": bias_np,
            "ws": ws_np,
        }
        for i in range(N_CORES)
    ]


def kernel(x, weight_packed, weight_scale, bias):
    in_maps = make_in_maps(
        {"x": x, "weight_packed": weight_packed,
         "weight_scale": weight_scale, "bias": bias})
    res = run_bass_kernel_spmd(_get_nc(), in_maps, list(range(N_CORES))).results
    y = np.concatenate([res[i]["y"] for i in range(N_CORES)], axis=0)
    return np.ascontiguousarray(y.reshape(B, S, O))
